# revision 10
# baseline (speedup 1.0000x reference)
"""Two-layer GCN (multi-label) on 8 Trainium2 NeuronCores.

Sharding: nodes padded to 8*NB; core k owns dst block k (graph parallel by
dst).  Host builds, as sharding metadata, per-core dst-sorted edge lists
split by src sub-table (int16 dma_gather limit), padded so all 8 cores share
ONE compile-time schedule (SPMD), plus fp8 one-hot scatter matrices, int16
gather indices and degree histograms.

Device per core:
  A) h1 = (feat * norm_src) @ W1 for ALL nodes (replicated, bf16 PE) ->
     4 per-group DRAM tables [25088,128] bf16 (so layer-1 gathers for group
     g start as soon as that quarter of the projection lands).
  B) layer-1 aggregation: dma_gather (4 SWDGE queues) fetches h1[src] rows
     in dst-tile order; PE accumulates S^T @ msg (S = fp8 one-hot of local
     dst) into PSUM per dst tile; epilogue relu(agg*norm_dst+b1) = x2, PE
     transpose, x2 @ W2 * norm_src -> h2s half-block tensors (f32).
  C) two chunked AllGathers (halves of the block, 49 tiles each) -> layer-2
     table in piece-major layout; layer-2 gathers for a sub-table start as
     soon as its collective lands.
  D) layer-2 aggregation identically (own idx/S in piece-major grouping;
     gathered f32 msgs cast to bf16); sigmoid epilogue -> out block f32.
Host concatenates the 8 blocks and slices to [n_nodes, n_cls].
"""
import sys
import numpy as np

sys.path.insert(0, "/opt/trn_rl_repo")

import ml_dtypes  # noqa: E402
import concourse.bass as bass  # noqa: E402
import concourse.mybir as mybir  # noqa: E402
import concourse.tile as tile  # noqa: E402
from concourse import bacc, bass2jax  # noqa: E402
from concourse.masks import make_identity  # noqa: E402

N_CORES = 8
P = 128
N_GROUPS = 4
MAX_ROWS = 2048  # per dma_gather instruction

BF16 = mybir.dt.bfloat16
F32 = mybir.dt.float32
FP8 = mybir.dt.float8e4
I16 = mybir.dt.int16


# ----------------------------------------------------------------------------
# host-side preprocessing
# ----------------------------------------------------------------------------

def _wrap_idx(flat):
    """dma_gather idx layout: idx i -> partition i%16, col i//16, replicated
    across the 8 16-partition groups."""
    w = len(flat) // 16
    return np.tile(flat.reshape(w, 16).T, (8, 1)).astype(np.int16)


def _prep_layer(src, dst, nb, grp_fn, idx_fn):
    """Uniform (SPMD) metadata for one aggregation layer.

    Returns idx_all [N_CORES][128, W] int16 (group-major layout),
    s_all [N_CORES][128, C*128] fp8 (tile-major), sched (shared).
    """
    ntiles = nb // P
    segs = []  # per core: {(t,g): (idx_arr, dstloc_arr)}
    for k in range(N_CORES):
        lo, hi = k * nb, (k + 1) * nb
        sel = (dst >= lo) & (dst < hi)
        es, ed = src[sel], dst[sel] - lo
        order = np.argsort(ed, kind="stable")
        es, ed = es[order], ed[order]
        tile_of = ed // P
        grp = grp_fn(es)
        loc = idx_fn(es)
        d = {}
        bounds = np.searchsorted(tile_of, np.arange(ntiles + 1))
        for t in range(ntiles):
            a, b = bounds[t], bounds[t + 1]
            gt = grp[a:b]
            for g in range(N_GROUPS):
                m = gt == g
                if m.any():
                    d[(t, g)] = (loc[a:b][m].astype(np.int32),
                                 (ed[a:b][m] - t * P).astype(np.int32))
        segs.append(d)

    seg_len = {}
    for t in range(ntiles):
        for g in range(N_GROUPS):
            n = max(len(segs[k].get((t, g), ((), ()))[0]) for k in range(N_CORES))
            if n:
                seg_len[(t, g)] = -(-n // P) * P
        if not any((t, g) in seg_len for g in range(N_GROUPS)):
            seg_len[(t, 0)] = P

    # group-major idx arrays: all tiles of group 0, then group 1, ...
    idx_cols = [[] for _ in range(N_CORES)]
    idx_off = {}  # (t, g) -> col offset (16-idx cols)
    off = 0
    for g in range(N_GROUPS):
        for t in range(ntiles):
            if (t, g) not in seg_len:
                continue
            L = seg_len[(t, g)]
            idx_off[(t, g)] = off
            for k in range(N_CORES):
                idx, _ = segs[k].get((t, g), (np.zeros(0, np.int32), None))
                flat = np.zeros(L, np.int32)
                flat[:len(idx)] = idx
                idx_cols[k].append(_wrap_idx(flat))
            off += L // 16

    # tile-major S blocks + per-tile chunk group lists
    s_blocks = [[] for _ in range(N_CORES)]
    tile_chunks = []  # per tile: list of (g, chunk_within_seg)
    iota = np.arange(P, dtype=np.int32)
    for t in range(ntiles):
        chunks = []
        for g in range(N_GROUPS):
            if (t, g) not in seg_len:
                continue
            L = seg_len[(t, g)]
            nch = L // P
            for k in range(N_CORES):
                _, dl = segs[k].get((t, g), (None, np.zeros(0, np.int32)))
                full = np.full(L, -1, np.int32)
                full[:len(dl)] = dl
                sb = (full[:, None] == iota[None, :]) \
                    .astype(ml_dtypes.float8_e4m3).reshape(nch, P, P)
                for c in range(nch):
                    s_blocks[k].append(sb[c])
            for c in range(nch):
                chunks.append((g, c))
        tile_chunks.append(chunks)

    # batches: consecutive tiles, per-group rows <= MAX_ROWS
    batches = []
    t0 = 0
    while t0 < ntiles:
        t1 = t0 + 1
        while t1 < ntiles:
            if any(sum(seg_len.get((t, g), 0) for t in range(t0, t1 + 1)) > MAX_ROWS
                   for g in range(N_GROUPS)):
                break
            t1 += 1
        batches.append((t0, t1))
        t0 = t1

    sched = []
    for (t0, t1) in batches:
        gathers = []
        msg_col = {}
        for g in range(N_GROUPS):
            keys = [(t, g) for t in range(t0, t1) if (t, g) in seg_len]
            if not keys:
                continue
            rows = sum(seg_len[key] for key in keys)
            col = 0
            for key in keys:
                msg_col[key] = col
                col += seg_len[key] // P
            gathers.append((g, idx_off[keys[0]], rows))
        tiles = []
        for t in range(t0, t1):
            ch = [(g, msg_col[(t, g)] + c) for (g, c) in tile_chunks[t]]
            tiles.append((t, ch))
        sched.append({"gathers": gathers, "tiles": tiles})

    idx_all = [np.ascontiguousarray(np.concatenate(c, axis=1)) for c in idx_cols]
    s_all = [np.ascontiguousarray(np.concatenate(blks, axis=1))
             for blks in s_blocks]
    return idx_all, s_all, sched


# ----------------------------------------------------------------------------
# device kernel
# ----------------------------------------------------------------------------

def _norms_from_deg(nc, pool, deg_tile, w, name):
    m = pool.tile([P, w], F32, name=f"{name}_m")
    nc.vector.tensor_scalar(out=m[:], in0=deg_tile[:], scalar1=1.0, scalar2=None,
                            op0=mybir.AluOpType.max)
    nc.scalar.sqrt(m[:], m[:])
    r = pool.tile([P, w], F32, name=f"{name}_r")
    nc.vector.reciprocal(r[:], m[:])
    g = pool.tile([P, w], F32, name=f"{name}_g")
    nc.vector.tensor_scalar(out=g[:], in0=deg_tile[:], scalar1=1.0, scalar2=None,
                            op0=mybir.AluOpType.min)
    nc.vector.tensor_mul(out=r[:], in0=r[:], in1=g[:])
    return r


def build_kernel(n_pad, nb, in_f, h_f, n_cls, sub, sched1, sched2,
                 idx1_w, s1_w, idx2_w, s2_w):
    nc = bacc.Bacc("TRN2", target_bir_lowering=False, debug=False,
                   num_devices=N_CORES, num_swdge_queues=4)
    ntiles = nb // P
    nkt = in_f // P
    half = nb // 2  # rows per h2s half (tile-aligned)

    feat_t = nc.declare_dram_parameter("feat_t", [nkt, P, n_pad], BF16,
                                       isOutput=False)
    w1 = nc.declare_dram_parameter("w1", [P, nkt, h_f], BF16, isOutput=False)
    w2 = nc.declare_dram_parameter("w2", [P, n_cls], BF16, isOutput=False)
    b1 = nc.declare_dram_parameter("b1", [P, h_f], F32, isOutput=False)
    b2 = nc.declare_dram_parameter("b2", [P, n_cls], F32, isOutput=False)
    degd = nc.declare_dram_parameter("degd", [P, ntiles], F32, isOutput=False)
    degs = nc.declare_dram_parameter("degs", [P, ntiles], F32, isOutput=False)
    idx1_in = nc.declare_dram_parameter("idx1", [P, idx1_w], I16, isOutput=False)
    s1_in = nc.declare_dram_parameter("s1", [P, s1_w], FP8, isOutput=False)
    idx2_in = nc.declare_dram_parameter("idx2", [P, idx2_w], I16, isOutput=False)
    s2_in = nc.declare_dram_parameter("s2", [P, s2_w], FP8, isOutput=False)
    out = nc.declare_dram_parameter("out", [nb, n_cls], F32, isOutput=True)

    with tile.TileContext(nc) as tc:
        with tc.tile_pool(name="const", bufs=1) as cst, \
             tc.tile_pool(name="dram", bufs=1, space="DRAM") as dram:

            idx1_sb = cst.tile([P, idx1_w], I16)
            nc.sync.dma_start(out=idx1_sb[:], in_=idx1_in[:])
            idx2_sb = cst.tile([P, idx2_w], I16)
            nc.sync.dma_start(out=idx2_sb[:], in_=idx2_in[:])
            w1_sb = cst.tile([P, nkt, h_f], BF16)
            nc.sync.dma_start(out=w1_sb[:], in_=w1[:])
            w2_sb = cst.tile([P, n_cls], BF16)
            nc.sync.dma_start(out=w2_sb[:], in_=w2[:])
            b1_bc = cst.tile([P, h_f], F32)
            nc.sync.dma_start(out=b1_bc[:], in_=b1[:])
            b2_bc = cst.tile([P, n_cls], F32)
            nc.sync.dma_start(out=b2_bc[:], in_=b2[:])
            degd_sb = cst.tile([P, ntiles], F32)
            nc.sync.dma_start(out=degd_sb[:], in_=degd[:])
            degs_sb = cst.tile([P, ntiles], F32)
            nc.sync.dma_start(out=degs_sb[:], in_=degs[:])
            norm_dst = _norms_from_deg(nc, cst, degd_sb, ntiles, "nd")
            norm_src = _norms_from_deg(nc, cst, degs_sb, ntiles, "ns")
            ident_bf = cst.tile([P, P], BF16)
            make_identity(nc, ident_bf[:])

            h1_tabs = [dram.tile([sub, h_f], BF16, name=f"h1g{g}")
                       for g in range(N_GROUPS)]
            h2s_half = [dram.tile([half, n_cls], F32, name=f"h2sh{j}")
                        for j in range(2)]
            h2_piece = [dram.tile([half * N_CORES, n_cls], F32,
                                  addr_space="Shared", name=f"h2p{j}")
                        for j in range(2)]

            # ---- phase A: projection (all nodes, replicated) ----
            GRP = 1024
            assert n_pad % GRP == 0 and sub % 256 == 0
            with tc.tile_pool(name="pa_sb", bufs=3) as pa, \
                 tc.tile_pool(name="pa_ps", bufs=2, space="PSUM") as pap, \
                 tc.tile_pool(name="pa_out", bufs=3) as pao:
                for gi in range(n_pad // GRP):
                    ft = pa.tile([P, nkt, GRP], BF16, tag="ft")
                    for ks in range(nkt):
                        nc.sync.dma_start(
                            out=ft[:, ks, :],
                            in_=feat_t[ks, :, gi * GRP:(gi + 1) * GRP])
                    for hh in range(GRP // 256):
                        ps_e = pap.tile([P, h_f], F32, space="PSUM", tag="pse")
                        ps_o = pap.tile([P, h_f], F32, space="PSUM", tag="pso")
                        base = hh * 256
                        for ks in range(nkt):
                            nc.tensor.matmul(
                                ps_e[:], lhsT=ft[:, ks, base:base + 256:2],
                                rhs=w1_sb[:, ks, :],
                                start=(ks == 0), stop=(ks == nkt - 1))
                        for ks in range(nkt):
                            nc.tensor.matmul(
                                ps_o[:], lhsT=ft[:, ks, base + 1:base + 256:2],
                                rhs=w1_sb[:, ks, :],
                                start=(ks == 0), stop=(ks == nkt - 1))
                        hb = pao.tile([P, 2, h_f], BF16, tag="hb")
                        nc.vector.tensor_copy(hb[:, 0, :], ps_e[:])
                        nc.scalar.copy(hb[:, 1, :], ps_o[:])
                        r0 = gi * GRP + hh * 256
                        tab = h1_tabs[r0 // sub]
                        rl = r0 % sub
                        nc.sync.dma_start(
                            out=tab[rl:rl + 256, :].rearrange(
                                "(p a) d -> p (a d)", a=2),
                            in_=hb[:].rearrange("p a d -> p (a d)"))

            # ---- aggregation layers ----
            def agg_layer(tables, d, tab_dt, idx_sb, s_in, sched, epilogue, tag):
                s_off = 0
                qi = 0
                with tc.tile_pool(name=f"msg_{tag}", bufs=3) as msgp, \
                     tc.tile_pool(name=f"s_{tag}", bufs=3) as sp, \
                     tc.tile_pool(name=f"ps_{tag}", bufs=2, space="PSUM") as psp, \
                     tc.tile_pool(name=f"ep_{tag}", bufs=3) as epp, \
                     tc.tile_pool(name=f"ept_{tag}", bufs=2, space="PSUM") as ept:
                    for binfo in sched:
                        msgs = {}
                        for (g, coff, rows) in binfo["gathers"]:
                            mt = msgp.tile([P, rows // P, d], tab_dt,
                                           tag=f"m{g}")
                            nc.gpsimd.dma_gather(
                                out_ap=mt[:],
                                in_ap=tables[g],
                                idxs_ap=idx_sb[:, coff:coff + rows // 16],
                                num_idxs=rows,
                                num_idxs_reg=rows,
                                elem_size=d,
                                single_packet=(rows <= 1024),
                                queue_num=qi % 4,
                            )
                            qi += 1
                            if tab_dt != BF16:
                                mb = msgp.tile([P, rows // P, d], BF16,
                                               tag=f"mb{g}")
                                nc.vector.tensor_copy(mb[:], mt[:])
                                msgs[g] = mb
                            else:
                                msgs[g] = mt
                        nch = sum(len(ch) for _, ch in binfo["tiles"])
                        st = sp.tile([P, nch * P], FP8, tag="sb")
                        nc.sync.dma_start(
                            out=st[:], in_=s_in[:, s_off:s_off + nch * P])
                        ci = 0
                        for (t, chunks) in binfo["tiles"]:
                            ps = psp.tile([P, d], F32, space="PSUM", tag="agg")
                            for j, (g, mcol) in enumerate(chunks):
                                nc.tensor.matmul(
                                    ps[:],
                                    lhsT=st[:, ci * P:(ci + 1) * P],
                                    rhs=msgs[g][:, mcol, :],
                                    start=(j == 0), stop=(j == len(chunks) - 1))
                                ci += 1
                            epilogue(t, ps, epp, ept)
                        s_off += nch * P

            def l1_epilogue(t, ps, epp, ept):
                t1 = epp.tile([P, h_f], F32, tag="t1")
                nc.vector.tensor_scalar(out=t1[:], in0=ps[:],
                                        scalar1=norm_dst[:, t:t + 1],
                                        scalar2=None,
                                        op0=mybir.AluOpType.mult)
                nc.vector.tensor_add(out=t1[:], in0=t1[:], in1=b1_bc[:])
                x2 = epp.tile([P, h_f], BF16, tag="x2")
                nc.scalar.activation(x2[:], t1[:],
                                     mybir.ActivationFunctionType.Relu)
                x2t_ps = ept.tile([P, h_f], BF16, space="PSUM", tag="x2t")
                nc.tensor.transpose(x2t_ps[:], x2[:], ident_bf[:])
                x2t = epp.tile([P, h_f], BF16, tag="x2ts")
                nc.vector.tensor_copy(x2t[:], x2t_ps[:])
                p2 = ept.tile([P, n_cls], F32, space="PSUM", tag="p2")
                nc.tensor.matmul(p2[:], lhsT=x2t[:], rhs=w2_sb[:],
                                 start=True, stop=True)
                h2 = epp.tile([P, n_cls], F32, tag="h2")
                nc.scalar.activation(h2[:], p2[:],
                                     mybir.ActivationFunctionType.Copy,
                                     bias=0.0, scale=norm_src[:, t:t + 1])
                j = (t * P) // half
                rl = (t * P) % half
                nc.sync.dma_start(out=h2s_half[j][rl:rl + P, :], in_=h2[:])

            def l2_epilogue(t, ps, epp, ept):
                t1 = epp.tile([P, n_cls], F32, tag="t1")
                nc.vector.tensor_scalar(out=t1[:], in0=ps[:],
                                        scalar1=norm_dst[:, t:t + 1],
                                        scalar2=None,
                                        op0=mybir.AluOpType.mult)
                nc.vector.tensor_add(out=t1[:], in0=t1[:], in1=b2_bc[:])
                o = epp.tile([P, n_cls], F32, tag="o")
                nc.scalar.activation(o[:], t1[:],
                                     mybir.ActivationFunctionType.Sigmoid)
                nc.sync.dma_start(out=out[t * P:(t + 1) * P, :], in_=o[:])

            agg_layer(h1_tabs, h_f, BF16, idx1_sb, s1_in, sched1,
                      l1_epilogue, "l1")

            for j in range(2):
                nc.gpsimd.collective_compute(
                    "AllGather", mybir.AluOpType.bypass,
                    replica_groups=[list(range(N_CORES))],
                    ins=[h2s_half[j].opt()],
                    outs=[h2_piece[j].opt()],
                )

            l2_tables = [h2_piece[0][0:sub, :], h2_piece[0][sub:2 * sub, :],
                         h2_piece[1][0:sub, :], h2_piece[1][sub:2 * sub, :]]
            agg_layer(l2_tables, n_cls, F32, idx2_sb, s2_in, sched2,
                      l2_epilogue, "l2")

    nc.compile()
    return nc


# ----------------------------------------------------------------------------
# entry
# ----------------------------------------------------------------------------

def _run(feat, src, dst, W1, b1, W2, b2, nb, sub, build_only=False):
    n_nodes, in_f = feat.shape
    h_f = W1.shape[1]
    n_cls = W2.shape[1]
    n_pad = nb * N_CORES
    half = nb // 2

    feat = np.asarray(feat, np.float32)
    src = np.asarray(src, np.int32)
    dst = np.asarray(dst, np.int32)

    deg_src = np.bincount(src, minlength=n_pad).astype(np.float32)
    deg_dst = np.bincount(dst, minlength=n_pad).astype(np.float32)
    norm_src = np.where(deg_src > 0,
                        1.0 / np.sqrt(np.maximum(deg_src, 1.0)), 0.0)

    feat_pad = np.zeros((n_pad, in_f), np.float32)
    feat_pad[:n_nodes] = feat * norm_src[:n_nodes, None]
    nkt = in_f // P
    feat_t = np.ascontiguousarray(
        feat_pad.T.reshape(nkt, P, n_pad)).astype(ml_dtypes.bfloat16)

    # layer 1: global grouping by src block of 'sub'
    idx1, s1, sched1 = _prep_layer(
        src, dst, nb,
        grp_fn=lambda v: v // sub,
        idx_fn=lambda v: v % sub)

    # layer 2: piece-major layout.  v = core c, offset o; half j = o//half;
    # table row = j*(4 halves... row within piece) ; piece j holds, core-major,
    # each core's half j.  Global sub-table g = j*2 + (c>=4).
    def l2_row(v):
        c, o = v // nb, v % nb
        return (o // half) * (half * N_CORES) + c * half + (o % half)

    idx2, s2, sched2 = _prep_layer(
        src, dst, nb,
        grp_fn=lambda v: l2_row(v) // sub,
        idx_fn=lambda v: l2_row(v) % sub)

    ntiles = nb // P
    w1_dev = np.ascontiguousarray(
        np.asarray(W1, np.float32).reshape(nkt, P, h_f).transpose(1, 0, 2)
    ).astype(ml_dtypes.bfloat16)
    w2_dev = np.asarray(W2, np.float32).astype(ml_dtypes.bfloat16)

    ins = []
    for k in range(N_CORES):
        lo = k * nb
        ins.append({
            "feat_t": feat_t,
            "w1": w1_dev,
            "w2": w2_dev,
            "b1": np.tile(np.asarray(b1, np.float32).reshape(1, h_f), (P, 1)),
            "b2": np.tile(np.asarray(b2, np.float32).reshape(1, n_cls), (P, 1)),
            "degd": np.ascontiguousarray(
                deg_dst[lo:lo + nb].reshape(ntiles, P).T, np.float32),
            "degs": np.ascontiguousarray(
                deg_src[lo:lo + nb].reshape(ntiles, P).T, np.float32),
            "idx1": idx1[k], "s1": s1[k],
            "idx2": idx2[k], "s2": s2[k],
        })

    nc = build_kernel(n_pad, nb, in_f, h_f, n_cls, sub, sched1, sched2,
                      idx1[0].shape[1], s1[0].shape[1],
                      idx2[0].shape[1], s2[0].shape[1])
    if build_only:
        return None, nc, ins
    outs = bass2jax.run_bass_via_pjrt(nc, ins, n_cores=N_CORES)
    full = np.concatenate([outs[k]["out"] for k in range(N_CORES)], axis=0)
    return full[:n_nodes].astype(np.float32), nc, ins


def kernel(feat, src, dst, W1, b1, W2, b2):
    n_nodes = feat.shape[0]
    if n_nodes == 100000:
        nb, sub = 12544, 25088
    else:  # small test configs
        nb = -(-(n_nodes // N_CORES) // P) * P
        sub = nb * N_CORES // N_GROUPS
    res, _, _ = _run(feat, src, dst, W1, b1, W2, b2, nb, sub)
    return res


# revision 16
# speedup vs baseline: 1.0954x; 1.0954x over previous
"""Two-layer GCN (multi-label) on 8 Trainium2 NeuronCores.

Sharding: nodes padded to 8*NB; core k owns dst block k (graph parallel by
dst).  Host builds, as sharding metadata, per-core dst-sorted edge lists
split by src sub-table (int16 dma_gather limit), padded so all 8 cores share
ONE compile-time schedule (SPMD), plus fp8 one-hot scatter matrices, int16
gather indices and degree histograms.

Device per core:
  A) h1 = (feat * norm_src) @ W1 for ALL nodes (replicated, bf16 PE) ->
     4 per-group DRAM tables [25088,128] bf16 (so layer-1 gathers for group
     g start as soon as that quarter of the projection lands).
  B) layer-1 aggregation: dma_gather (4 SWDGE queues) fetches h1[src] rows
     in dst-tile order; PE accumulates S^T @ msg (S = fp8 one-hot of local
     dst) into PSUM per dst tile; epilogue relu(agg*norm_dst+b1) = x2, PE
     transpose, x2 @ W2 * norm_src -> h2s half-block tensors (f32).
  C) two chunked AllGathers (halves of the block, 49 tiles each) -> layer-2
     table in piece-major layout; layer-2 gathers for a sub-table start as
     soon as its collective lands.
  D) layer-2 aggregation identically (own idx/S in piece-major grouping;
     gathered f32 msgs cast to bf16); sigmoid epilogue -> out block f32.
Host concatenates the 8 blocks and slices to [n_nodes, n_cls].
"""
import sys
import numpy as np

sys.path.insert(0, "/opt/trn_rl_repo")

import ml_dtypes  # noqa: E402
import concourse.bass as bass  # noqa: E402
import concourse.mybir as mybir  # noqa: E402
import concourse.tile as tile  # noqa: E402
from concourse import bacc, bass2jax  # noqa: E402
from concourse.masks import make_identity  # noqa: E402

N_CORES = 8
P = 128
N_GROUPS = 4
MAX_ROWS = 2048  # per dma_gather instruction

BF16 = mybir.dt.bfloat16
F32 = mybir.dt.float32
FP8 = mybir.dt.float8e4
I16 = mybir.dt.int16


# ----------------------------------------------------------------------------
# host-side preprocessing
# ----------------------------------------------------------------------------

def _wrap_idx(flat):
    """dma_gather idx layout: idx i -> partition i%16, col i//16, replicated
    across the 8 16-partition groups."""
    w = len(flat) // 16
    return np.tile(flat.reshape(w, 16).T, (8, 1)).astype(np.int16)


def _prep_layer(src, dst, nb, grp_fn, idx_fn):
    """Uniform (SPMD) metadata for one aggregation layer.

    Returns idx_all [N_CORES][128, W] int16 (group-major layout),
    s_all [N_CORES][128, C*128] fp8 (tile-major), sched (shared).
    """
    ntiles = nb // P
    segs = []  # per core: {(t,g): (idx_arr, dstloc_arr)}
    for k in range(N_CORES):
        lo, hi = k * nb, (k + 1) * nb
        sel = (dst >= lo) & (dst < hi)
        es, ed = src[sel], dst[sel] - lo
        order = np.argsort(ed, kind="stable")
        es, ed = es[order], ed[order]
        tile_of = ed // P
        grp = grp_fn(es)
        loc = idx_fn(es)
        d = {}
        bounds = np.searchsorted(tile_of, np.arange(ntiles + 1))
        for t in range(ntiles):
            a, b = bounds[t], bounds[t + 1]
            gt = grp[a:b]
            for g in range(N_GROUPS):
                m = gt == g
                if m.any():
                    d[(t, g)] = (loc[a:b][m].astype(np.int32),
                                 (ed[a:b][m] - t * P).astype(np.int32))
        segs.append(d)

    seg_len = {}
    for t in range(ntiles):
        for g in range(N_GROUPS):
            n = max(len(segs[k].get((t, g), ((), ()))[0]) for k in range(N_CORES))
            if n:
                seg_len[(t, g)] = -(-n // P) * P
        if not any((t, g) in seg_len for g in range(N_GROUPS)):
            seg_len[(t, 0)] = P

    # group-major idx arrays: all tiles of group 0, then group 1, ...
    idx_cols = [[] for _ in range(N_CORES)]
    idx_off = {}  # (t, g) -> col offset (16-idx cols)
    off = 0
    for g in range(N_GROUPS):
        for t in range(ntiles):
            if (t, g) not in seg_len:
                continue
            L = seg_len[(t, g)]
            idx_off[(t, g)] = off
            for k in range(N_CORES):
                idx, _ = segs[k].get((t, g), (np.zeros(0, np.int32), None))
                flat = np.zeros(L, np.int32)
                flat[:len(idx)] = idx
                idx_cols[k].append(_wrap_idx(flat))
            off += L // 16

    # tile-major S blocks + per-tile chunk group lists
    s_blocks = [[] for _ in range(N_CORES)]
    tile_chunks = []  # per tile: list of (g, chunk_within_seg)
    iota = np.arange(P, dtype=np.int32)
    for t in range(ntiles):
        chunks = []
        for g in range(N_GROUPS):
            if (t, g) not in seg_len:
                continue
            L = seg_len[(t, g)]
            nch = L // P
            for k in range(N_CORES):
                _, dl = segs[k].get((t, g), (None, np.zeros(0, np.int32)))
                full = np.full(L, -1, np.int32)
                full[:len(dl)] = dl
                sb = (full[:, None] == iota[None, :]) \
                    .astype(ml_dtypes.float8_e4m3).reshape(nch, P, P)
                for c in range(nch):
                    s_blocks[k].append(sb[c])
            for c in range(nch):
                chunks.append((g, c))
        tile_chunks.append(chunks)

    # batches: consecutive tiles, per-group rows <= MAX_ROWS
    batches = []
    t0 = 0
    while t0 < ntiles:
        t1 = t0 + 1
        while t1 < ntiles:
            if any(sum(seg_len.get((t, g), 0) for t in range(t0, t1 + 1)) > MAX_ROWS
                   for g in range(N_GROUPS)):
                break
            t1 += 1
        batches.append((t0, t1))
        t0 = t1

    sched = []
    for (t0, t1) in batches:
        gathers = []
        msg_col = {}
        for g in range(N_GROUPS):
            keys = [(t, g) for t in range(t0, t1) if (t, g) in seg_len]
            if not keys:
                continue
            rows = sum(seg_len[key] for key in keys)
            col = 0
            for key in keys:
                msg_col[key] = col
                col += seg_len[key] // P
            gathers.append((g, idx_off[keys[0]], rows))
        tiles = []
        for t in range(t0, t1):
            ch = [(g, msg_col[(t, g)] + c) for (g, c) in tile_chunks[t]]
            tiles.append((t, ch))
        sched.append({"gathers": gathers, "tiles": tiles})

    idx_all = [np.ascontiguousarray(np.concatenate(c, axis=1)) for c in idx_cols]
    s_all = [np.ascontiguousarray(np.concatenate(blks, axis=1))
             for blks in s_blocks]
    return idx_all, s_all, sched


# ----------------------------------------------------------------------------
# device kernel
# ----------------------------------------------------------------------------

def _norms_from_deg(nc, pool, deg_tile, w, name):
    m = pool.tile([P, w], F32, name=f"{name}_m")
    nc.vector.tensor_scalar(out=m[:], in0=deg_tile[:], scalar1=1.0, scalar2=None,
                            op0=mybir.AluOpType.max)
    nc.scalar.sqrt(m[:], m[:])
    r = pool.tile([P, w], F32, name=f"{name}_r")
    nc.vector.reciprocal(r[:], m[:])
    g = pool.tile([P, w], F32, name=f"{name}_g")
    nc.vector.tensor_scalar(out=g[:], in0=deg_tile[:], scalar1=1.0, scalar2=None,
                            op0=mybir.AluOpType.min)
    nc.vector.tensor_mul(out=r[:], in0=r[:], in1=g[:])
    return r


def build_kernel(n_pad, nb, in_f, h_f, n_cls, sub, sched1, sched2,
                 idx1_w, s1_w, idx2_w, s2_w):
    nc = bacc.Bacc("TRN2", target_bir_lowering=False, debug=False,
                   num_devices=N_CORES, num_swdge_queues=4)
    ntiles = nb // P
    nkt = in_f // P
    half = nb // 2  # rows per h2s half (tile-aligned)

    feat_t = nc.declare_dram_parameter("feat_t", [P, nkt, n_pad], BF16,
                                       isOutput=False)
    w1 = nc.declare_dram_parameter("w1", [P, nkt, h_f], BF16, isOutput=False)
    w2 = nc.declare_dram_parameter("w2", [P, n_cls], BF16, isOutput=False)
    b1 = nc.declare_dram_parameter("b1", [P, h_f], F32, isOutput=False)
    b2 = nc.declare_dram_parameter("b2", [P, n_cls], F32, isOutput=False)
    degd = nc.declare_dram_parameter("degd", [P, ntiles], F32, isOutput=False)
    degs = nc.declare_dram_parameter("degs", [P, ntiles], F32, isOutput=False)
    idx1_in = nc.declare_dram_parameter("idx1", [P, idx1_w], I16, isOutput=False)
    s1_in = nc.declare_dram_parameter("s1", [P, s1_w], FP8, isOutput=False)
    idx2_in = nc.declare_dram_parameter("idx2", [P, idx2_w], I16, isOutput=False)
    s2_in = nc.declare_dram_parameter("s2", [P, s2_w], FP8, isOutput=False)
    out = nc.declare_dram_parameter("out", [nb, n_cls], F32, isOutput=True)

    with tile.TileContext(nc) as tc:
        with tc.tile_pool(name="const", bufs=1) as cst, \
             tc.tile_pool(name="dram", bufs=1, space="DRAM") as dram:

            idx1_sb = cst.tile([P, idx1_w], I16)
            nc.sync.dma_start(out=idx1_sb[:], in_=idx1_in[:])
            idx2_sb = cst.tile([P, idx2_w], I16)
            nc.sync.dma_start(out=idx2_sb[:], in_=idx2_in[:])
            w1_sb = cst.tile([P, nkt, h_f], BF16)
            nc.sync.dma_start(out=w1_sb[:], in_=w1[:])
            w2_sb = cst.tile([P, n_cls], BF16)
            nc.sync.dma_start(out=w2_sb[:], in_=w2[:])
            b1_bc = cst.tile([P, h_f], F32)
            nc.sync.dma_start(out=b1_bc[:], in_=b1[:])
            b2_bc = cst.tile([P, n_cls], F32)
            nc.sync.dma_start(out=b2_bc[:], in_=b2[:])
            degd_sb = cst.tile([P, ntiles], F32)
            nc.sync.dma_start(out=degd_sb[:], in_=degd[:])
            degs_sb = cst.tile([P, ntiles], F32)
            nc.sync.dma_start(out=degs_sb[:], in_=degs[:])
            norm_dst = _norms_from_deg(nc, cst, degd_sb, ntiles, "nd")
            norm_src = _norms_from_deg(nc, cst, degs_sb, ntiles, "ns")
            ident_bf = cst.tile([P, P], BF16)
            make_identity(nc, ident_bf[:])

            h1_tabs = [dram.tile([sub, h_f], BF16, name=f"h1g{g}")
                       for g in range(N_GROUPS)]
            h2s_half = [dram.tile([half, n_cls], F32, name=f"h2sh{j}")
                        for j in range(2)]
            h2_piece = [dram.tile([half * N_CORES, n_cls], F32,
                                  addr_space="Shared", name=f"h2p{j}")
                        for j in range(2)]

            # ---- phase A: projection (all nodes, replicated) ----
            # 512 nodes per iteration: one read DMA (sync), 4 interleaved
            # 128-node matmuls, one write DMA (scalar) of 4 paired tiles so
            # every DMA moves >=1KB/partition at full descriptor rate.
            GRP = 512
            assert n_pad % GRP == 0 and sub % GRP == 0
            with tc.tile_pool(name="pa_sb", bufs=3) as pa, \
                 tc.tile_pool(name="pa_ps", bufs=3, space="PSUM") as pap, \
                 tc.tile_pool(name="pa_out", bufs=3) as pao:
                for gi in range(n_pad // GRP):
                    ft = pa.tile([P, nkt, GRP], BF16, tag="ft")
                    nc.sync.dma_start(
                        out=ft[:], in_=feat_t[:, :, gi * GRP:(gi + 1) * GRP])
                    hb = pao.tile([P, 4, h_f], BF16, tag="hb")
                    for j in range(4):
                        ps = pap.tile([P, h_f], F32, space="PSUM", tag="pp")
                        for ks in range(nkt):
                            nc.tensor.matmul(
                                ps[:], lhsT=ft[:, ks, j::4],
                                rhs=w1_sb[:, ks, :],
                                start=(ks == 0), stop=(ks == nkt - 1))
                        nc.vector.tensor_copy(hb[:, j, :], ps[:])
                    r0 = gi * GRP
                    tab = h1_tabs[r0 // sub]
                    rl = r0 % sub
                    nc.scalar.dma_start(
                        out=tab[rl:rl + GRP, :].rearrange(
                            "(p a) d -> p (a d)", a=4),
                        in_=hb[:].rearrange("p a d -> p (a d)"))

            # ---- aggregation layers ----
            def agg_layer(tables, d, tab_dt, idx_sb, s_in, sched, epilogue, tag,
                          post_batch=None):
                s_off = 0
                qi = 0
                with tc.tile_pool(name=f"msg_{tag}", bufs=3) as msgp, \
                     tc.tile_pool(name=f"s_{tag}", bufs=3) as sp, \
                     tc.tile_pool(name=f"ps_{tag}", bufs=2, space="PSUM") as psp, \
                     tc.tile_pool(name=f"ep_{tag}", bufs=3) as epp, \
                     tc.tile_pool(name=f"ept_{tag}", bufs=2, space="PSUM") as ept:
                    for binfo in sched:
                        msgs = {}
                        for (g, coff, rows) in binfo["gathers"]:
                            mt = msgp.tile([P, rows // P, d], tab_dt,
                                           tag=f"m{g}")
                            nc.gpsimd.dma_gather(
                                out_ap=mt[:],
                                in_ap=tables[g],
                                idxs_ap=idx_sb[:, coff:coff + rows // 16],
                                num_idxs=rows,
                                num_idxs_reg=rows,
                                elem_size=d,
                                single_packet=(rows <= 1024),
                                queue_num=qi % 4,
                            )
                            qi += 1
                            if tab_dt != BF16:
                                mb = msgp.tile([P, rows // P, d], BF16,
                                               tag=f"mb{g}")
                                nc.vector.tensor_copy(mb[:], mt[:])
                                msgs[g] = mb
                            else:
                                msgs[g] = mt
                        nch = sum(len(ch) for _, ch in binfo["tiles"])
                        st = sp.tile([P, nch * P], FP8, tag="sb")
                        nc.sync.dma_start(
                            out=st[:], in_=s_in[:, s_off:s_off + nch * P])
                        ci = 0
                        for (t, chunks) in binfo["tiles"]:
                            ps = psp.tile([P, d], F32, space="PSUM", tag="agg")
                            for j, (g, mcol) in enumerate(chunks):
                                nc.tensor.matmul(
                                    ps[:],
                                    lhsT=st[:, ci * P:(ci + 1) * P],
                                    rhs=msgs[g][:, mcol, :],
                                    start=(j == 0), stop=(j == len(chunks) - 1))
                                ci += 1
                            epilogue(t, ps, epp, ept)
                        s_off += nch * P
                        if post_batch is not None:
                            post_batch(binfo["tiles"][-1][0])

            def l1_epilogue(t, ps, epp, ept):
                t1 = epp.tile([P, h_f], F32, tag="t1")
                nc.vector.tensor_scalar(out=t1[:], in0=ps[:],
                                        scalar1=norm_dst[:, t:t + 1],
                                        scalar2=None,
                                        op0=mybir.AluOpType.mult)
                nc.vector.tensor_add(out=t1[:], in0=t1[:], in1=b1_bc[:])
                x2 = epp.tile([P, h_f], BF16, tag="x2")
                nc.scalar.activation(x2[:], t1[:],
                                     mybir.ActivationFunctionType.Relu)
                x2t_ps = ept.tile([P, h_f], BF16, space="PSUM", tag="x2t")
                nc.tensor.transpose(x2t_ps[:], x2[:], ident_bf[:])
                x2t = epp.tile([P, h_f], BF16, tag="x2ts")
                nc.vector.tensor_copy(x2t[:], x2t_ps[:])
                p2 = ept.tile([P, n_cls], F32, space="PSUM", tag="p2")
                nc.tensor.matmul(p2[:], lhsT=x2t[:], rhs=w2_sb[:],
                                 start=True, stop=True)
                h2 = epp.tile([P, n_cls], F32, tag="h2")
                nc.scalar.activation(h2[:], p2[:],
                                     mybir.ActivationFunctionType.Copy,
                                     bias=0.0, scale=norm_src[:, t:t + 1])
                j = (t * P) // half
                rl = (t * P) % half
                nc.sync.dma_start(out=h2s_half[j][rl:rl + P, :], in_=h2[:])

            def l2_epilogue(t, ps, epp, ept):
                t1 = epp.tile([P, n_cls], F32, tag="t1")
                nc.vector.tensor_scalar(out=t1[:], in0=ps[:],
                                        scalar1=norm_dst[:, t:t + 1],
                                        scalar2=None,
                                        op0=mybir.AluOpType.mult)
                nc.vector.tensor_add(out=t1[:], in0=t1[:], in1=b2_bc[:])
                o = epp.tile([P, n_cls], F32, tag="o")
                nc.scalar.activation(o[:], t1[:],
                                     mybir.ActivationFunctionType.Sigmoid)
                nc.sync.dma_start(out=out[t * P:(t + 1) * P, :], in_=o[:])

            # interleave the half-block AllGathers into the L1 batch loop so
            # each fires (on the collective engine) as soon as its half of the
            # epilogues is written, overlapping the remaining L1 desc-gen.
            coll_state = {"next": 0}
            half_tiles = half // P

            def fire_collectives(last_tile):
                while (coll_state["next"] < 2
                       and last_tile >= (coll_state["next"] + 1) * half_tiles - 1):
                    j = coll_state["next"]
                    nc.gpsimd.collective_compute(
                        "AllGather", mybir.AluOpType.bypass,
                        replica_groups=[list(range(N_CORES))],
                        ins=[h2s_half[j].opt()],
                        outs=[h2_piece[j].opt()],
                    )
                    coll_state["next"] += 1

            agg_layer(h1_tabs, h_f, BF16, idx1_sb, s1_in, sched1,
                      l1_epilogue, "l1", post_batch=fire_collectives)
            fire_collectives(ntiles)  # safety: any not yet fired

            l2_tables = [h2_piece[0][0:sub, :], h2_piece[0][sub:2 * sub, :],
                         h2_piece[1][0:sub, :], h2_piece[1][sub:2 * sub, :]]
            agg_layer(l2_tables, n_cls, F32, idx2_sb, s2_in, sched2,
                      l2_epilogue, "l2")

    nc.compile()
    return nc


# ----------------------------------------------------------------------------
# entry
# ----------------------------------------------------------------------------

def _run(feat, src, dst, W1, b1, W2, b2, nb, sub, build_only=False):
    n_nodes, in_f = feat.shape
    h_f = W1.shape[1]
    n_cls = W2.shape[1]
    n_pad = nb * N_CORES
    half = nb // 2

    feat = np.asarray(feat, np.float32)
    src = np.asarray(src, np.int32)
    dst = np.asarray(dst, np.int32)

    deg_src = np.bincount(src, minlength=n_pad).astype(np.float32)
    deg_dst = np.bincount(dst, minlength=n_pad).astype(np.float32)
    norm_src = np.where(deg_src > 0,
                        1.0 / np.sqrt(np.maximum(deg_src, 1.0)), 0.0)

    feat_pad = np.zeros((n_pad, in_f), np.float32)
    feat_pad[:n_nodes] = feat * norm_src[:n_nodes, None]
    nkt = in_f // P
    feat_t = np.ascontiguousarray(
        feat_pad.T.reshape(nkt, P, n_pad).transpose(1, 0, 2)
    ).astype(ml_dtypes.bfloat16)

    # layer 1: global grouping by src block of 'sub'
    idx1, s1, sched1 = _prep_layer(
        src, dst, nb,
        grp_fn=lambda v: v // sub,
        idx_fn=lambda v: v % sub)

    # layer 2: piece-major layout.  v = core c, offset o; half j = o//half;
    # table row = j*(4 halves... row within piece) ; piece j holds, core-major,
    # each core's half j.  Global sub-table g = j*2 + (c>=4).
    def l2_row(v):
        c, o = v // nb, v % nb
        return (o // half) * (half * N_CORES) + c * half + (o % half)

    idx2, s2, sched2 = _prep_layer(
        src, dst, nb,
        grp_fn=lambda v: l2_row(v) // sub,
        idx_fn=lambda v: l2_row(v) % sub)

    ntiles = nb // P
    w1_dev = np.ascontiguousarray(
        np.asarray(W1, np.float32).reshape(nkt, P, h_f).transpose(1, 0, 2)
    ).astype(ml_dtypes.bfloat16)
    w2_dev = np.asarray(W2, np.float32).astype(ml_dtypes.bfloat16)

    ins = []
    for k in range(N_CORES):
        lo = k * nb
        ins.append({
            "feat_t": feat_t,
            "w1": w1_dev,
            "w2": w2_dev,
            "b1": np.tile(np.asarray(b1, np.float32).reshape(1, h_f), (P, 1)),
            "b2": np.tile(np.asarray(b2, np.float32).reshape(1, n_cls), (P, 1)),
            "degd": np.ascontiguousarray(
                deg_dst[lo:lo + nb].reshape(ntiles, P).T, np.float32),
            "degs": np.ascontiguousarray(
                deg_src[lo:lo + nb].reshape(ntiles, P).T, np.float32),
            "idx1": idx1[k], "s1": s1[k],
            "idx2": idx2[k], "s2": s2[k],
        })

    nc = build_kernel(n_pad, nb, in_f, h_f, n_cls, sub, sched1, sched2,
                      idx1[0].shape[1], s1[0].shape[1],
                      idx2[0].shape[1], s2[0].shape[1])
    if build_only:
        return None, nc, ins
    outs = bass2jax.run_bass_via_pjrt(nc, ins, n_cores=N_CORES)
    full = np.concatenate([outs[k]["out"] for k in range(N_CORES)], axis=0)
    return full[:n_nodes].astype(np.float32), nc, ins


def kernel(feat, src, dst, W1, b1, W2, b2):
    n_nodes = feat.shape[0]
    if n_nodes == 100000:
        nb, sub = 12544, 25088
    else:  # small test configs
        nb = -(-(n_nodes // N_CORES) // P) * P
        sub = nb * N_CORES // N_GROUPS
    res, _, _ = _run(feat, src, dst, W1, b1, W2, b2, nb, sub)
    return res
